# revision 3
# baseline (speedup 1.0000x reference)
"""DeepSeekMoE forward on 8 Trainium2 NeuronCores (expert-parallel).

Strategy (per the sharding hint):
  - Gate softmax + top-2 routing computed on host (tiny: [1024, 8]).
  - Host plays the all-to-all: tokens are gathered per expert, padded to a
    common capacity C, and shipped (transposed) to the expert's core.
  - Core e computes its routed expert's SwiGLU on its C tokens, plus a
    1/8 slice (176 of 1408 intermediate channels) of the shared expert
    over ALL tokens. Shared-expert partials sum across cores.
  - Host un-permutes routed outputs (scaled by normalized top-k weights),
    sums shared partials, computes the aux loss.

All matmuls run as float32r (full-rate fp32 path on the PE, ~1e-4 matmul
relative error) with fp32 PSUM accumulation.
"""

import os

_jp = os.environ.get("JAX_PLATFORMS")
if _jp is not None and "axon" not in _jp:
    # The Bass program executes through the axon PJRT tunnel; never let a
    # caller-pinned JAX_PLATFORMS=cpu hide the neuron devices.
    os.environ["JAX_PLATFORMS"] = "axon"

import numpy as np
import orjson

import concourse.bass as bass
import concourse.mybir as mybir
import concourse.tile as tile
from concourse.bass_utils import run_bass_kernel_spmd

# ---------------------------------------------------------------------------
# Workaround: this container's walrus build rejects >1 sync wait per
# instruction ("Too many sync wait commands"), but Tile's scheduler attaches
# several (tail Drain, LDWEIGHTS with two DMA deps...). Patch the single
# serialization choke point to hoist extra waits onto EventSemaphore
# instructions inserted just before the owner on the same engine (engine
# streams are in-order, so gating earlier is equivalent).
# ---------------------------------------------------------------------------

_wsplit_counter = [0]


def _fix_obj(obj):
    if isinstance(obj, dict):
        for v in obj.values():
            _fix_obj(v)
        return
    if not isinstance(obj, list):
        return
    if not any(
        isinstance(it, dict) and "opcode" in it and "engine" in it for it in obj
    ):
        for it in obj:
            _fix_obj(it)
        return
    i = 0
    while i < len(obj):
        inst = obj[i]
        if isinstance(inst, dict) and "opcode" in inst and "engine" in inst:
            si = inst.get("sync_info")
            waits = (si.get("on_wait") or []) if si else []
            if len(waits) > 1:
                si["on_wait"] = [waits[0]]
                inserts = []
                for w in waits[1:]:
                    _wsplit_counter[0] += 1
                    inserts.append(
                        {
                            "debug": inst.get("debug", 0),
                            "engine": inst["engine"],
                            "ins": [],
                            "name": f"wsplit_{_wsplit_counter[0]}",
                            "opcode": "EventSemaphore",
                            "outs": [],
                            "sync_info": {"on_update": [], "on_wait": [w]},
                        }
                    )
                obj[i:i] = inserts
                i += len(inserts)
        else:
            _fix_obj(inst)
        i += 1


_orig_to_json_bytes = bass.Bass.to_json_bytes


def _patched_to_json_bytes(self) -> bytes:
    doc = orjson.loads(_orig_to_json_bytes(self))
    _fix_obj(doc)
    return orjson.dumps(doc)


bass.Bass.to_json_bytes = _patched_to_json_bytes

# ---------------------------------------------------------------------------
# Problem constants (hardcoded per the task contract).
# ---------------------------------------------------------------------------

TOP_K = 2
AUX_ALPHA = 0.001
H = 2048  # hidden
M = 1408  # routed-expert intermediate
E = 8  # routed experts == n_cores
MSH = M // E  # shared-expert intermediate slice per core (176)

KT = H // 128  # 16 contraction tiles
MT = M // 128  # 11 intermediate tiles
SHB = 256  # shared-expert token block
F32 = mybir.dt.float32
F32R = mybir.dt.float32r
SILU = mybir.ActivationFunctionType.Silu


def _build(C: int, CC: int, N: int) -> bass.Bass:
    """One core's program: routed expert on C gathered tokens (processed in
    chunks of CC <= 512) + a 176-row shared-expert slice on all N tokens.
    Inputs are pre-transposed so the contraction dim lands on SBUF
    partitions with contiguous DMA rows."""
    assert N % SHB == 0 and C % CC == 0
    NB = N // SHB
    nc = bass.Bass()

    xt = nc.dram_tensor("xt", [H, N], F32R, kind="ExternalInput")
    xr = nc.dram_tensor("xr", [H, C], F32R, kind="ExternalInput")
    wg = nc.dram_tensor("wg", [H, M], F32R, kind="ExternalInput")
    wu = nc.dram_tensor("wu", [H, M], F32R, kind="ExternalInput")
    wd = nc.dram_tensor("wd", [M, H], F32R, kind="ExternalInput")
    sg = nc.dram_tensor("sg", [H, MSH], F32R, kind="ExternalInput")
    su = nc.dram_tensor("su", [H, MSH], F32R, kind="ExternalInput")
    sd = nc.dram_tensor("sd", [MSH, H], F32R, kind="ExternalInput")
    yr = nc.dram_tensor("yr", [H, C], F32, kind="ExternalOutput")
    ys = nc.dram_tensor("ys", [H, N], F32, kind="ExternalOutput")

    xt_t = xt.rearrange("(k p) n -> p k n", p=128)  # [128, KT, N]
    xr_t = xr.rearrange("(k p) c -> p k c", p=128)  # [128, KT, C]
    wg_t = wg.rearrange("(k p) m -> p k m", p=128)  # [128, KT, M]
    wu_t = wu.rearrange("(k p) m -> p k m", p=128)
    wd_t = wd.rearrange("(t p) h -> p t h", p=128)  # [128, MT, H]
    sg_t = sg.rearrange("(k p) m -> p k m", p=128)  # [128, KT, MSH]
    su_t = su.rearrange("(k p) m -> p k m", p=128)
    yr_t = yr.rearrange("(t p) c -> t p c", p=128)  # [KT, 128, C]
    ys_t = ys.rearrange("(t p) n -> t p n", p=128)

    with tile.TileContext(nc) as tc:
        with (
            tc.tile_pool(name="resident", bufs=1) as res,
            tc.tile_pool(name="xblk", bufs=2) as xbp,
            tc.tile_pool(name="weights", bufs=2) as wpool,
            tc.tile_pool(name="acts", bufs=2) as apool,
            tc.tile_pool(name="outs", bufs=3) as opool,
            tc.tile_pool(name="psum", bufs=2, space="PSUM") as ps,
        ):
            # ---- shared expert slice over all N tokens ----
            sg_s = res.tile([128, KT, MSH], F32R)
            nc.sync.dma_start(out=sg_s, in_=sg_t)
            su_s = res.tile([128, KT, MSH], F32R)
            nc.sync.dma_start(out=su_s, in_=su_t)
            sd0_s = res.tile([128, KT, 128], F32R)
            nc.sync.dma_start(
                out=sd0_s, in_=sd[0:128, :].rearrange("p (t h) -> p t h", h=128)
            )
            sd1_s = res.tile([MSH - 128, KT, 128], F32R)
            nc.sync.dma_start(
                out=sd1_s, in_=sd[128:MSH, :].rearrange("p (t h) -> p t h", h=128)
            )

            for nb in range(NB):
                sl = slice(nb * SHB, (nb + 1) * SHB)
                xb = xbp.tile([128, KT, SHB], F32R, tag="xb")
                nc.sync.dma_start(out=xb, in_=xt_t[:, :, sl])
                a0 = apool.tile([128, SHB], F32R, tag="a0")
                a1 = apool.tile([MSH - 128, SHB], F32R, tag="a1")
                for msub, (mp, a_sh) in enumerate(((128, a0), (MSH - 128, a1))):
                    pg = ps.tile([mp, SHB], F32, tag="pg")
                    pu = ps.tile([mp, SHB], F32, tag="pu")
                    msl = slice(msub * 128, msub * 128 + mp)
                    for k in range(KT):
                        nc.tensor.matmul(
                            pg, sg_s[:, k, msl], xb[:, k, :],
                            start=(k == 0), stop=(k == KT - 1),
                        )
                    for k in range(KT):
                        nc.tensor.matmul(
                            pu, su_s[:, k, msl], xb[:, k, :],
                            start=(k == 0), stop=(k == KT - 1),
                        )
                    nc.scalar.activation(a_sh, pg, SILU)
                    nc.vector.tensor_mul(a_sh, a_sh, pu)
                for ht in range(KT):
                    py = ps.tile([128, SHB], F32, tag="py")
                    nc.tensor.matmul(
                        py, sd0_s[:, ht, :], a0, start=True, stop=False
                    )
                    nc.tensor.matmul(
                        py, sd1_s[:, ht, :], a1, start=False, stop=True
                    )
                    yo = opool.tile([128, SHB], F32, tag="yo")
                    nc.vector.tensor_copy(yo, py)
                    nc.sync.dma_start(out=ys_t[ht, :, sl], in_=yo)

            # ---- routed expert on C gathered tokens, chunks of CC ----
            xr_s = res.tile([128, KT, C], F32R)
            nc.sync.dma_start(out=xr_s, in_=xr_t)
            for ch in range(C // CC):
                csl = slice(ch * CC, (ch + 1) * CC)
                a_all = apool.tile([128, MT, CC], F32R, tag="a_all")
                for mt in range(MT):
                    msl = slice(mt * 128, (mt + 1) * 128)
                    wgt = wpool.tile([128, KT, 128], F32R, tag="wgt")
                    nc.sync.dma_start(out=wgt, in_=wg_t[:, :, msl])
                    wut = wpool.tile([128, KT, 128], F32R, tag="wut")
                    nc.sync.dma_start(out=wut, in_=wu_t[:, :, msl])
                    pg = ps.tile([128, CC], F32, tag="pg")
                    pu = ps.tile([128, CC], F32, tag="pu")
                    for k in range(KT):
                        nc.tensor.matmul(
                            pg, wgt[:, k, :], xr_s[:, k, csl],
                            start=(k == 0), stop=(k == KT - 1),
                        )
                    for k in range(KT):
                        nc.tensor.matmul(
                            pu, wut[:, k, :], xr_s[:, k, csl],
                            start=(k == 0), stop=(k == KT - 1),
                        )
                    nc.scalar.activation(a_all[:, mt, :], pg, SILU)
                    nc.vector.tensor_mul(a_all[:, mt, :], a_all[:, mt, :], pu)
                for ht in range(KT):
                    wdt = wpool.tile([128, MT, 128], F32R, tag="wdt")
                    nc.sync.dma_start(
                        out=wdt, in_=wd_t[:, :, ht * 128 : (ht + 1) * 128]
                    )
                    py = ps.tile([128, CC], F32, tag="py")
                    for mt in range(MT):
                        nc.tensor.matmul(
                            py, wdt[:, mt, :], a_all[:, mt, :],
                            start=(mt == 0), stop=(mt == MT - 1),
                        )
                    yo = opool.tile([128, CC], F32, tag="yo")
                    nc.vector.tensor_copy(yo, py)
                    nc.sync.dma_start(out=yr_t[ht, :, csl], in_=yo)

    return nc


_cache: dict = {}


def kernel(
    hidden_states,
    gate_weight,
    w_gate,
    w_up,
    w_down,
    sh_gate,
    sh_up,
    sh_down,
):
    hs = np.asarray(hidden_states, np.float32)
    b, s, h = hs.shape
    assert h == H
    x = hs.reshape(-1, H)
    n = x.shape[0]
    gate_w = np.asarray(gate_weight, np.float32)
    w_gate = np.asarray(w_gate, np.float32)
    w_up = np.asarray(w_up, np.float32)
    w_down = np.asarray(w_down, np.float32)
    sh_gate = np.asarray(sh_gate, np.float32)
    sh_up = np.asarray(sh_up, np.float32)
    sh_down = np.asarray(sh_down, np.float32)

    # ---- gate: softmax + top-2 (host; matches jax.lax.top_k tie-breaks) ----
    logits = x @ gate_w.T
    mx = logits.max(axis=-1, keepdims=True)
    ex = np.exp(logits - mx)
    scores = ex / ex.sum(axis=-1, keepdims=True)
    order = np.argsort(-scores, axis=1, kind="stable")
    top2 = order[:, :TOP_K]
    tw = np.take_along_axis(scores, top2, axis=1)
    twn = tw / (tw.sum(axis=-1, keepdims=True) + 1e-20)

    # ---- aux loss (seq_aux over B sequences) ----
    idx_flat = top2.reshape(b, -1)
    pi = scores.reshape(b, s, E).mean(axis=1).astype(np.float64)
    aux = 0.0
    for bi in range(b):
        ce = np.bincount(idx_flat[bi], minlength=E).astype(np.float64)
        ce /= s * TOP_K / E
        aux += (ce * pi[bi]).sum()
    aux = np.float32(aux / b * AUX_ALPHA)

    # ---- dispatch (host plays the all-to-all) ----
    rows_per_e = []
    wts_per_e = []
    for e in range(E):
        r, which = np.nonzero(top2 == e)
        rows_per_e.append(r)
        wts_per_e.append(twn[r, which])
    cap = max(1, max(len(r) for r in rows_per_e))
    nch = -(-cap // 512)  # chunks of <=512 tokens (PSUM bank / fp32 limit)
    CC = -(-cap // (nch * 64)) * 64
    C = nch * CC

    key = (C, CC, n)
    nc = _cache.get(key)
    if nc is None:
        nc = _build(C, CC, n)
        _cache[key] = nc

    xt = np.ascontiguousarray(x.T)
    in_maps = []
    for e in range(E):
        r = rows_per_e[e]
        xre = np.zeros((H, C), np.float32)
        if len(r):
            xre[:, : len(r)] = x[r].T
        lo, hi = e * MSH, (e + 1) * MSH
        in_maps.append(
            {
                "xt": xt,
                "xr": xre,
                "wg": np.ascontiguousarray(w_gate[e].T),
                "wu": np.ascontiguousarray(w_up[e].T),
                "wd": np.ascontiguousarray(w_down[e].T),
                "sg": np.ascontiguousarray(sh_gate[lo:hi].T),
                "su": np.ascontiguousarray(sh_up[lo:hi].T),
                "sd": np.ascontiguousarray(sh_down[:, lo:hi].T),
            }
        )

    res = run_bass_kernel_spmd(nc, in_maps, core_ids=list(range(E)))

    # ---- combine (host plays the gather side of the all-to-all) ----
    y = np.zeros((n, H), np.float64)
    for e in range(E):
        out = res.results[e]
        y += out["ys"].T
        r = rows_per_e[e]
        if len(r):
            # rows are unique within one expert (a token picks an expert
            # at most once), so fancy-index += is safe
            y[r] += out["yr"][:, : len(r)].T * wts_per_e[e][:, None]
    return y.astype(np.float32).reshape(b, s, H), aux
